# revision 88
# baseline (speedup 1.0000x reference)
"""Trainium2 Bass kernel for PointCloudTeacher (2x EdgeConv with KNN graph).

Sharding: 8 NeuronCores, B=4 point clouds of N=4096 points; core c handles
(batch b = c//2, row-half h = c%2) = 2048 query rows. Per-core inputs are
row-PERMUTED so the core's own half comes first -- one SPMD program serves
all cores. Two launches with a host gather of x1 in between.

Per block:
  - Coarse KNN keys via single-pass TF32 (float32r) gram matmul plus a K=1
    rank-1 pass adding the (mean-centered) -|x_m|^2/2 bias. Block2 scans
    top-8 per 1024-wide quarter (pipelined against the gram) and merges via
    a position-mask; block1 scans the full 4096 row at once.
  - Exact re-rank of coarse ranks 1..5: gather candidate rows (with the
    exact bias column), DVE multiply+reduce against the center row [x_n, 1]
    -> exact f32 keys. The self key is host-precomputed in an xa column.
  - y_a = (s*W_a) @ x for the full cloud in the main loop, paced one tile
    per KNN iteration (block2 pre-emits 8 to cover the x^T load, both keep
    a tail batch to cover the KNN drain); y_c = (s*W_c) @ x + b' for own
    rows is computed inside the EPILOGUE loop straight from PSUM (no DRAM
    round-trip), overlapping the epilogue gathers.
  - Epilogue: self row of y_a direct-loaded + 4 indirect gathers by the
    exact top-4 indices, max over k via a DVE pairwise tree, add y_c,
    LeakyReLU via Prelu(alpha=0.2).
Precision: block1 y-matmuls use a 3-pass tf32+bf16 hi/lo split (x1 feeds
KNN2's exact rerank so it needs ~f32 quality); block2 y-matmuls are bf16
with bf16 y_a storage and bf16 output (only faces the 2e-2 gate).
Queue discipline: bulk loads ride SP in priority order; y_a/xout stores and
cent loads ride the Act queue -- keeps dependency-stalled writes from
head-of-line-blocking the SP stripe prefetches.
"""

import numpy as np
import ml_dtypes
from contextlib import ExitStack

import concourse.bass as bass
import concourse.bacc as bacc
import concourse.mybir as mybir
from concourse.tile import TileContext
from concourse.bass_utils import run_bass_kernel_spmd

dt = mybir.dt
AF = mybir.ActivationFunctionType
OP = mybir.AluOpType

P = 128
N = 4096
HALF = 2048
B = 4
C1, O1 = 512, 864
C2, O2 = 864, 1728
K = 5
NCAND = 8        # max8 output width
SHORT = 6        # rerank shortlist size (coarse top-6 covers exact top-5)
EPS = 1e-5
SLOPE = 0.2
XA1 = 516   # rerank row width block1: x row + bias + pad
XA2 = 868   # rerank row width block2: x1 row + bias + pad
# y_a schedule split points (pre-emitted / end-batch tile counts)
_PRE1, _END1 = 0, 16
_PRE2, _END2 = 8, 10
N_TILES = N // P        # 32
H_TILES = HALF // P     # 16


# ---------------------------------------------------------------- host utils

def _tf32(a):
    a = np.ascontiguousarray(a, dtype=np.float32)
    u = a.view(np.uint32).astype(np.uint64)
    u = ((u + 0x1000 + ((u >> 13) & 1)) & 0xFFFFE000).astype(np.uint32)
    return u.view(np.float32)


def _bf16(a):
    return np.ascontiguousarray(a, dtype=np.float32).astype(ml_dtypes.bfloat16)


def _stripes(xT, n_k):
    """(cin, N) -> (N_TILES, 128, n_k*128) tile-major lhsT stripes.
    [t, c, ci*128+col] = xT[ci*128+c, t*128+col]; rows past cin are zero."""
    cin, n = xT.shape
    out = np.zeros((N_TILES, P, n_k * P), xT.dtype)
    for ci in range(n_k):
        kk = min(P, cin - ci * P)
        blk = xT[ci * P:ci * P + kk, :].reshape(kk, N_TILES, P)
        out[:, :kk, ci * P:(ci + 1) * P] = blk.transpose(1, 0, 2)
    return out


def _mid_bcast(ap, rep):
    """(P, F) access pattern -> (P, rep, F) with 0-stride middle dim."""
    pat = [list(ap.ap[0]), [0, rep], list(ap.ap[1])]
    return bass.AP(ap.tensor, ap.offset, pat)


def _last_bcast(ap, rep):
    """(P, F) access pattern -> (P, F, rep) with 0-stride last dim."""
    pat = [list(ap.ap[0]), list(ap.ap[1]), [0, rep]]
    return bass.AP(ap.tensor, ap.offset, pat)


def _chunks(c):
    out = []
    o = 0
    while o < c:
        kk = min(P, c - o)
        out.append((o, kk))
        o += kk
    return out


# ---------------------------------------------------------------- program

def _build_block(cin, cout, xa_w, split_y):
    """split_y=True: block1 (3-pass tf32+bf16 y-matmuls, f32 y_a table).
    split_y=False: block2 (bf16 y-matmuls, bf16 y_a table)."""
    nc = bacc.Bacc("TRN2", target_bir_lowering=False)

    ksizes = _chunks(cin)
    n_k = len(ksizes)
    ytile = dt.float32 if split_y else dt.bfloat16

    # ---- inputs
    xTr = nc.dram_tensor("xTr", [cin + 2, N], dt.float32r, kind="ExternalInput")
    xa = nc.dram_tensor("xa", [N, xa_w], dt.float32, kind="ExternalInput")
    io8 = nc.dram_tensor("io8", [P, NCAND], dt.float32, kind="ExternalInput")
    io32 = nc.dram_tensor("io32", [P, 4 * NCAND], dt.float32,
                          kind="ExternalInput")
    qoff = nc.dram_tensor("qoff", [P, 4 * NCAND], dt.float32,
                          kind="ExternalInput")
    ones1 = nc.dram_tensor("ones1", [1, P], dt.float32r, kind="ExternalInput")
    if split_y:
        # bf16 lo/hi stripe streams for the cross passes
        xtl_s = nc.dram_tensor("xtl_s", [N_TILES, P, n_k * P], dt.bfloat16,
                               kind="ExternalInput")
        xthb_s = nc.dram_tensor("xthb_s", [N_TILES, P, n_k * P], dt.bfloat16,
                                kind="ExternalInput")
        wah = nc.dram_tensor("wah", [cin, cout], dt.float32r, kind="ExternalInput")
        wal = nc.dram_tensor("wal", [cin, cout], dt.bfloat16, kind="ExternalInput")
        wahb = nc.dram_tensor("wahb", [cin, cout], dt.bfloat16, kind="ExternalInput")
        wch = nc.dram_tensor("wch", [cin, cout], dt.float32r, kind="ExternalInput")
        wcl = nc.dram_tensor("wcl", [cin, cout], dt.bfloat16, kind="ExternalInput")
        wchb = nc.dram_tensor("wchb", [cin, cout], dt.bfloat16, kind="ExternalInput")
        b1b = nc.dram_tensor("b1b", [P, cout], dt.float32, kind="ExternalInput")
    else:
        # bf16 x1^T stripe stream for the y lhsT side
        x1b_s = nc.dram_tensor("x1b_s", [N_TILES, P, n_k * P], dt.bfloat16,
                               kind="ExternalInput")
        wah = nc.dram_tensor("wah", [cin, cout], dt.bfloat16, kind="ExternalInput")
        wch = nc.dram_tensor("wch", [cin + 1, cout], dt.bfloat16,
                             kind="ExternalInput")
        onesb = nc.dram_tensor("onesb", [1, P], dt.bfloat16, kind="ExternalInput")

    # internal dram + output. Block1's x1 feeds block2's exact rerank and
    # must stay f32; block2's output only faces the 2e-2 gate, so bf16.
    yad = nc.dram_tensor("yad", [N, cout], ytile)
    xout = nc.dram_tensor("xout", [HALF, cout],
                          dt.float32 if split_y else dt.bfloat16,
                          kind="ExternalOutput")

    osub = [(o, min(512, cout - o)) for o in range(0, cout, 512)]
    if cout == O2:
        # subtiles aligned so [0:864] and [864:1728] split into two psum tiles
        osub = [(0, 512), (512, 352), (864, 512), (1376, 352)]
    n_half = (cout + 863) // 864
    half_w = min(cout, 864)

    with TileContext(nc) as tc:
        prog = ExitStack()
        pp = prog.enter_context(tc.tile_pool(name="persist", bufs=1))
        pidx = prog.enter_context(tc.tile_pool(name="pidx", bufs=H_TILES))
        # weights stay resident for the epilogue y_c pass
        pw = prog.enter_context(tc.tile_pool(name="pw", bufs=1))
        if split_y:
            # block1 epilogue y_c needs the f32r x^T chunks as lhsT
            pxtr = prog.enter_context(tc.tile_pool(name="pxtr", bufs=1))

        # ---- weight tiles. Block1: both groups resident. Block2: the c-group
        # is loaded in the epilogue phase (after the big x^T chunks free).
        def load_w(grp, pool, eng_hi=None, eng_lo=None):
            eng_hi = eng_hi or nc.sync
            eng_lo = eng_lo or eng_hi
            wsrc = wah if grp == "a" else wch
            wdt = dt.float32r if split_y else dt.bfloat16
            wh_t, wl_t, whb_t = [], [], []
            for ci, (off, kk) in enumerate(ksizes):
                t = pool.tile([kk, cout], wdt, tag=f"w{grp}h{ci}")
                eng_hi.dma_start(t[:], wsrc[off:off + kk, :])
                wh_t.append(t)
                if split_y:
                    wsl = wal if grp == "a" else wcl
                    wsb = wahb if grp == "a" else wchb
                    t = pool.tile([kk, cout], dt.bfloat16, tag=f"w{grp}l{ci}")
                    eng_lo.dma_start(t[:], wsl[off:off + kk, :])
                    wl_t.append(t)
                    t = pool.tile([kk, cout], dt.bfloat16, tag=f"w{grp}hb{ci}")
                    eng_lo.dma_start(t[:], wsb[off:off + kk, :])
                    whb_t.append(t)
            return wh_t, wl_t, whb_t

        wcbias = None

        def y_matmuls(pshs, tile, grp, trio, stripe_pool, xtr_tiles):
            """emit the matmul stream for one (128, cout) y tile."""
            cs = slice(tile * P, (tile + 1) * P)
            wh_t, wl_t, whb_t = trio
            if split_y:
                stl = stripe_pool.tile([P, n_k * P], dt.bfloat16, tag="stl")
                nc.sync.dma_start(stl[:], xtl_s[tile])
                sthb = stripe_pool.tile([P, n_k * P], dt.bfloat16, tag="sthb")
                nc.sync.dma_start(sthb[:], xthb_s[tile])
            else:
                stb = stripe_pool.tile([P, n_k * P], dt.bfloat16, tag="stb")
                nc.sync.dma_start(stb[:], x1b_s[tile])
            for (oo, ow) in osub:
                hh, po = (0, oo) if oo < 864 else (1, oo - 864)
                ps = pshs[hh]
                if split_y:
                    # bf16 stripe passes first: they only need the small
                    # weight/stripe loads, so pre-emitted y tiles can start
                    # while the f32r x^T chunks are still streaming in.
                    for ci, (off, kk) in enumerate(ksizes):
                        nc.tensor.matmul(
                            ps[:, po:po + ow],
                            lhsT=sthb[:kk, ci * P:(ci + 1) * P],
                            rhs=wl_t[ci][:, oo:oo + ow],
                            start=(ci == 0), stop=False,
                            skip_group_check=True,
                        )
                    for ci, (off, kk) in enumerate(ksizes):
                        nc.tensor.matmul(
                            ps[:, po:po + ow],
                            lhsT=stl[:kk, ci * P:(ci + 1) * P],
                            rhs=whb_t[ci][:, oo:oo + ow],
                            start=False, stop=False,
                            skip_group_check=True,
                        )
                    for ci, (off, kk) in enumerate(ksizes):
                        nc.tensor.matmul(
                            ps[:, po:po + ow],
                            lhsT=xtr_tiles[ci][:kk, cs],
                            rhs=wh_t[ci][:, oo:oo + ow],
                            start=False, stop=(ci == n_k - 1),
                            skip_group_check=True,
                        )
                else:
                    for ci, (off, kk) in enumerate(ksizes):
                        nc.tensor.matmul(
                            ps[:, po:po + ow],
                            lhsT=stb[:kk, ci * P:(ci + 1) * P],
                            rhs=wh_t[ci][:, oo:oo + ow],
                            start=(ci == 0),
                            stop=(grp == "a" and ci == n_k - 1),
                            skip_group_check=True,
                        )
                    if grp == "c":
                        nc.tensor.matmul(
                            ps[:, po:po + ow],
                            lhsT=onesbt[:],
                            rhs=wcbias[:, oo:oo + ow],
                            start=False, stop=True,
                            skip_group_check=True,
                        )

        idx4_t = []

        # ------------- main phase: KNN(t) interleaved with y_a tiles
        with (
            tc.tile_pool(name="pstr", bufs=2) as pstr,
            tc.tile_pool(name="pac", bufs=2) as pac,
            tc.tile_pool(name="pk", bufs=2 if split_y else 4) as pk,
            tc.tile_pool(name="pbs", bufs=2) as pbs,
            tc.tile_pool(name="pbc", bufs=2) as pbc,
            tc.tile_pool(name="pcent", bufs=2) as pcent,
            tc.tile_pool(name="pgq", bufs=2, space="PSUM") as pgq,
            tc.tile_pool(name="pyp", bufs=2, space="PSUM") as pyp,
        ):
            xst = ExitStack()
            if split_y:
                pxt = pxtr
            else:
                pxt = xst.enter_context(tc.tile_pool(name="pxt", bufs=1))

            def load_xtr(eng):
                """returns (chunk tiles, fn(nsl) -> bias-row AP)."""
                xtr_t = []
                for ci, (off, kk) in enumerate(ksizes):
                    t = pxt.tile([kk, N], dt.float32r, tag=f"xtr{ci}")
                    eng.dma_start(t[:], xTr[off:off + kk, :])
                    xtr_t.append(t)
                bt = pxt.tile([1, N], dt.float32r, tag="biasrow")
                eng.dma_start(bt[:], xTr[cin + 1:cin + 2, :])
                bias_ap = lambda nsl: bt[0:1, nsl]
                return xtr_t, bias_ap

            # All bulk loads ride the SP queue in priority order (the DMA
            # pipe is FIFO by issue, so queue order IS pipe priority).
            # Small constants ride Act. Block1's c-group weights are only
            # needed in the epilogue and load at the end of the main phase.
            onest = pp.tile([1, P], dt.float32r, tag="onest")
            nc.scalar.dma_start(onest[:], ones1[:])
            io8t = pp.tile([P, NCAND], dt.float32, tag="io8")
            nc.scalar.dma_start(io8t[:], io8[:])
            io32t = pp.tile([P, 4 * NCAND], dt.float32, tag="io32")
            nc.scalar.dma_start(io32t[:], io32[:])
            qofft = pp.tile([P, 4 * NCAND], dt.float32, tag="qoff")
            nc.scalar.dma_start(qofft[:], qoff[:])
            if split_y:
                xtr_t, bias_ap = load_xtr(nc.sync)
                wa_trio = load_w("a", pw, eng_hi=nc.sync)
            else:
                onesbt = pp.tile([1, P], dt.bfloat16, tag="onesbt")
                nc.scalar.dma_start(onesbt[:], onesb[:])
                wa_trio = load_w("a", pw, eng_hi=nc.sync)
                xtr_t = bias_ap = None  # loaded after the pre-emitted y_a

            def y_tile_a(tile):
                pshs = [pyp.tile([P, half_w], dt.float32, tag="yps",
                                 name=f"yps{hh}") for hh in range(n_half)]
                y_matmuls(pshs, tile, "a", wa_trio, pstr, xtr_t)
                cs = slice(tile * P, (tile + 1) * P)
                for hh, ps in enumerate(pshs):
                    sb = pac.tile([P, half_w], ytile, tag="ya_sb")
                    nc.scalar.activation(sb[:], ps[:], AF.Copy)
                    # write via the Act queue: a dependency-stalled write on
                    # SP would head-of-line-block the next stripe prefetch
                    nc.scalar.dma_start(
                        yad[cs, hh * 864:hh * 864 + half_w], sb[:]
                    )

            def knn_front(t):
                """gram keys -> coarse top8 -> gathers -> multiply/reduce.
                Block2 scans per quarter (pipelined against the gram, merged
                via a position-mask); block1's DVE is tighter, so it keeps
                the cheaper full-width scan."""
                cs = slice(t * P, (t + 1) * P)
                nq = 1 if split_y else 4
                qw = N // nq
                if nq > 1:
                    vcat = pbs.tile([P, 4 * NCAND], dt.float32, tag="vcat")
                    icat = pbs.tile([P, 4 * NCAND], dt.uint32, tag="icat")
                for q in range(nq):
                    keys = pk.tile([P, qw], dt.float32, tag="keys")
                    for qq in range(qw // 1024):
                        ps = pgq.tile([P, 1024], dt.float32, tag="gps")
                        for si in range(2):
                            n0 = q * qw + qq * 1024 + si * 512
                            nsl = slice(n0, n0 + 512)
                            psl = slice(si * 512, si * 512 + 512)
                            for ci, (off, kk) in enumerate(ksizes):
                                nc.tensor.matmul(
                                    ps[:, psl],
                                    lhsT=xtr_t[ci][:kk, cs],
                                    rhs=xtr_t[ci][:kk, nsl],
                                    start=(ci == 0), stop=False,
                                    skip_group_check=True,
                                )
                            nc.tensor.matmul(
                                ps[:, psl],
                                lhsT=onest[:],
                                rhs=bias_ap(nsl),
                                start=False, stop=True,
                                skip_group_check=True,
                            )
                        nc.scalar.activation(
                            keys[:, qq * 1024:(qq + 1) * 1024], ps[:], AF.Copy
                        )
                    if nq > 1:
                        qsl = slice(q * NCAND, (q + 1) * NCAND)
                        nc.vector.max(out=vcat[:, qsl], in_=keys[:])
                        nc.vector.max_index(icat[:, qsl], vcat[:, qsl],
                                            keys[:])
                if nq > 1:
                    # merge: global top8 of the 32 quarter-candidates
                    icatf = pbs.tile([P, 4 * NCAND], dt.float32, tag="icatf")
                    nc.vector.tensor_copy(icatf[:], icat[:])
                    nc.vector.tensor_tensor(out=icatf[:], in0=icatf[:],
                                            in1=qofft[:], op=OP.add)
                    mtop = pbs.tile([P, NCAND], dt.float32, tag="mtop")
                    nc.vector.max(out=mtop[:], in_=vcat[:])
                    mpos = pbs.tile([P, NCAND], dt.uint32, tag="mpos")
                    nc.vector.max_index(mpos[:], mtop[:], vcat[:])
                    mposf = pbs.tile([P, NCAND], dt.float32, tag="mposf")
                    nc.vector.tensor_copy(mposf[:], mpos[:])
                    m832 = pbs.tile([P, NCAND, 4 * NCAND], dt.float32,
                                    tag="m832")
                    nc.vector.tensor_tensor(
                        out=m832[:], in0=_mid_bcast(io32t[:], NCAND),
                        in1=_last_bcast(mposf[:], 4 * NCAND), op=OP.is_equal,
                    )
                    nc.vector.tensor_tensor(
                        out=m832[:], in0=m832[:],
                        in1=_mid_bcast(icatf[:], NCAND), op=OP.mult,
                    )
                    cidxf = pbs.tile([P, NCAND], dt.float32, tag="cidxf")
                    nc.vector.tensor_reduce(
                        out=cidxf[:], in_=m832[:], axis=mybir.AxisListType.X,
                        op=OP.add,
                    )
                    cidx = pbs.tile([P, NCAND], dt.uint32, tag="cidx")
                    nc.vector.tensor_copy(cidx[:], cidxf[:])
                else:
                    top8 = pbs.tile([P, NCAND], dt.float32, tag="top8")
                    nc.vector.max(out=top8[:], in_=keys[:])
                    cidx = pbs.tile([P, NCAND], dt.uint32, tag="cidx")
                    nc.vector.max_index(cidx[:], top8[:], keys[:])
                    cidxf = pbs.tile([P, NCAND], dt.float32, tag="cidxf")
                    nc.vector.tensor_copy(cidxf[:], cidx[:])

                # exact rerank: slot 0 = self key, precomputed on the host as
                # an extra xa column. Gathered candidates (coarse ranks 1..5)
                # in chunks of 3+2 so the gather->multiply->reduce chain
                # pipelines at sub-tile granularity.
                cent = pcent.tile([P, xa_w], dt.float32, tag="cent")
                nc.scalar.dma_start(cent[:], xa[cs, :])
                ekeys = pbs.tile([P, NCAND], dt.float32, tag="ekeys")
                nc.vector.memset(ekeys[:], -3e38)
                nc.vector.tensor_copy(ekeys[:, 0:1], cent[:, cin + 1:cin + 2])
                nc.vector.memset(cent[:, cin + 1:cin + 2], 0.0)
                nc.vector.memset(cent[:, cin:cin + 1], 1.0)
                CH = SHORT // 2
                for j0 in (1, 1 + CH):
                    ch = min(CH, SHORT - j0)
                    cand = pbc.tile([P, CH, xa_w], dt.float32, tag="cand")
                    for j in range(j0, j0 + ch):
                        nc.gpsimd.indirect_dma_start(
                            out=cand[:, j - j0, :],
                            out_offset=None,
                            in_=xa[:],
                            in_offset=bass.IndirectOffsetOnAxis(
                                ap=cidx[:, j:j + 1], axis=0
                            ),
                        )
                    # DVE multiply+reduce: cross-engine hops here lengthen
                    # the KNN chain latency more than DVE occupancy costs
                    nc.vector.tensor_tensor(
                        out=cand[:, :ch, :], in0=cand[:, :ch, :],
                        in1=_mid_bcast(cent[:], ch), op=OP.mult,
                    )
                    nc.vector.tensor_reduce(
                        out=ekeys[:, j0:j0 + ch], in_=cand[:, :ch, :],
                        axis=mybir.AxisListType.X, op=OP.add,
                    )
                return ekeys, cidxf

            def knn_back(st):
                """exact top-4 indices from the reranked keys."""
                ekeys, cidxf = st
                etop = pbs.tile([P, NCAND], dt.float32, tag="etop")
                nc.vector.max(out=etop[:], in_=ekeys[:])
                epos = pbs.tile([P, NCAND], dt.uint32, tag="epos")
                nc.vector.max_index(epos[:], etop[:], ekeys[:])
                eposf = pbs.tile([P, NCAND], dt.float32, tag="eposf")
                nc.vector.tensor_copy(eposf[:], epos[:])
                KR = K - 1
                m48 = pbs.tile([P, KR, NCAND], dt.float32, tag="m48")
                nc.vector.tensor_tensor(
                    out=m48[:], in0=_mid_bcast(io8t[:], KR),
                    in1=_last_bcast(eposf[:, 1:K], NCAND), op=OP.is_equal,
                )
                nc.vector.tensor_tensor(
                    out=m48[:], in0=m48[:], in1=_mid_bcast(cidxf[:], KR),
                    op=OP.mult,
                )
                idx4f = pbs.tile([P, KR], dt.float32, tag="idx4f")
                nc.vector.tensor_reduce(
                    out=idx4f[:], in_=m48[:], axis=mybir.AxisListType.X,
                    op=OP.add,
                )
                idx4 = pidx.tile([P, KR], dt.uint32, tag="idx4")
                nc.vector.tensor_copy(idx4[:], idx4f[:])
                idx4_t.append(idx4)

            # y_a schedule: block2 pre-emits tiles to cover the x^T load
            # window (block1's y_a needs x^T itself, so it leads with gram);
            # then one per iteration to pace with the KNN pipeline, and a
            # tail batch after the final gram to cover KNN drain + barrier.
            pre = _PRE1 if split_y else _PRE2
            for i in range(pre):
                y_tile_a(i)
            if not split_y:
                # x^T loads queue behind the pre-emitted stripes on SP
                xtr_t, bias_ap = load_xtr(nc.sync)
            lim = N_TILES - (_END1 if split_y else _END2)
            pending = None
            for t in range(H_TILES):
                st = knn_front(t)
                if pending is not None:
                    knn_back(pending)
                pending = st
                if pre + t < lim:
                    y_tile_a(pre + t)
            knn_back(pending)
            for yti in range(lim, N_TILES):
                y_tile_a(yti)
            if split_y:
                # epilogue weights, loaded behind everything else on SP
                wc_trio = load_w("c", pw, eng_hi=nc.sync)
                b1t = pw.tile([P, cout], dt.float32, tag="b1t")
                nc.sync.dma_start(b1t[:], b1b[:])

            xst.close()

        tc.strict_bb_all_engine_barrier()

        # ------------- epilogue phase: gathers + y_c + max + act
        with (
            tc.tile_pool(name="pg", bufs=2 if split_y else 3) as pg,
            tc.tile_pool(name="pe2", bufs=2 if split_y else 3) as pe2,
            tc.tile_pool(name="pstr2", bufs=2) as pstr2,
            tc.tile_pool(name="pwc", bufs=1) as pwc,
            tc.tile_pool(name="pyp2", bufs=4 if split_y else 2,
                         space="PSUM") as pyp2,
        ):
            if not split_y:
                wc_trio = load_w("c", pwc)
                wcbias = pwc.tile([1, cout], dt.bfloat16, tag="wcbias")
                nc.sync.dma_start(wcbias[:], wch[cin:cin + 1, :])
            for t in range(H_TILES):
                cs = slice(t * P, (t + 1) * P)
                # y_c first: its stripe prefetch beats the big g5 self-load
                # in the SP queue, keeping the PE fed
                pshs = [pyp2.tile([P, half_w], dt.float32, tag="ycps",
                                  name=f"ycps{hh}") for hh in range(n_half)]
                y_matmuls(pshs, t, "c", wc_trio, pstr2,
                          xtr_t if split_y else None)
                g5 = pg.tile([P, K, cout], ytile, tag="g5")
                nc.sync.dma_start(g5[:, 0, :], yad[cs, :])
                for j in range(K - 1):
                    nc.gpsimd.indirect_dma_start(
                        out=g5[:, j + 1, :],
                        out_offset=None,
                        in_=yad[:],
                        in_offset=bass.IndirectOffsetOnAxis(
                            ap=idx4_t[t][:, j:j + 1], axis=0
                        ),
                    )
                ycs = pe2.tile([P, cout], dt.float32, tag="ycs")
                for hh, ps in enumerate(pshs):
                    osl = slice(hh * 864, hh * 864 + half_w)
                    if split_y:
                        nc.vector.tensor_tensor(
                            out=ycs[:, osl], in0=ps[:], in1=b1t[:, osl],
                            op=OP.add,
                        )
                    else:
                        nc.scalar.activation(ycs[:, osl], ps[:], AF.Copy)
                # max over k: pairwise tree
                mb = pe2.tile([P, cout], ytile, tag="mb")
                nc.vector.tensor_tensor(out=mb[:], in0=g5[:, 0, :],
                                        in1=g5[:, 1, :], op=OP.max)
                for j in range(2, K):
                    nc.vector.tensor_tensor(out=mb[:], in0=mb[:],
                                            in1=g5[:, j, :], op=OP.max)
                xo = pe2.tile([P, cout], dt.float32, tag="xo")
                nc.vector.tensor_tensor(out=xo[:], in0=mb[:], in1=ycs[:],
                                        op=OP.add)
                if split_y:
                    nc.scalar.activation(xo[:], xo[:], AF.Prelu, alpha=SLOPE)
                    nc.scalar.dma_start(xout[cs, :], xo[:])
                else:
                    xo2 = pe2.tile([P, cout], dt.bfloat16, tag="xo2")
                    nc.scalar.activation(xo2[:], xo[:], AF.Prelu, alpha=SLOPE)
                    nc.scalar.dma_start(xout[cs, :], xo2[:])

        prog.close()

    nc.finalize()
    return nc


_CACHE = {}


def _get_programs():
    if "p1" not in _CACHE:
        _CACHE["p1"] = _build_block(C1, O1, XA1, split_y=True)
        _CACHE["p2"] = _build_block(C2, O2, XA2, split_y=False)
    return _CACHE["p1"], _CACHE["p2"]


# ---------------------------------------------------------------- host side

def _fold_bn(W, gamma, beta, mean, var, cin):
    s = gamma.astype(np.float64) / np.sqrt(var.astype(np.float64) + EPS)
    Wp = s[:, None] * W.astype(np.float64)
    Wa = Wp[:, :cin].T
    Wc = (Wp[:, cin:] - Wp[:, :cin]).T
    bp = beta.astype(np.float64) - s * mean.astype(np.float64)
    return (np.ascontiguousarray(Wa, np.float32),
            np.ascontiguousarray(Wc, np.float32),
            bp.astype(np.float32))


def _xtr_aug(xT, bias):
    bias_row = _tf32(np.asarray(bias, np.float32))[None, :]
    return np.concatenate(
        [_tf32(xT), np.ones((1, N), np.float32), bias_row], axis=0
    )


def _knn_bias(x):
    """mean-centered -|x|^2/2 (ranking-invariant, small tf32 ulp) and the
    exact self key (|x|^2 + mean)/2 for the rerank's slot 0."""
    sq = np.einsum("nc,nc->n", x.astype(np.float64), x.astype(np.float64))
    b = -(sq - sq.mean()) / 2
    sk = (sq + sq.mean()) / 2
    return b.astype(np.float32), sk.astype(np.float32)


def _prep_block1(x, Wa, Wc, bp):
    xT = np.ascontiguousarray(x.T)
    bias, selfk = _knn_bias(x)
    xTh = _tf32(xT)
    xa = np.zeros((N, XA1), np.float32)
    xa[:, :C1] = x
    xa[:, C1] = bias
    xa[:, C1 + 1] = selfk
    wah = _tf32(Wa)
    wch = _tf32(Wc)
    return dict(
        xTr=_xtr_aug(xT, bias),
        xtl_s=_stripes(_bf16(xT - xTh), len(_chunks(C1))),
        xthb_s=_stripes(_bf16(xTh), len(_chunks(C1))),
        xa=xa,
        wah=wah, wal=_bf16(Wa - wah), wahb=_bf16(wah),
        wch=wch, wcl=_bf16(Wc - wch), wchb=_bf16(wch),
        b1b=np.broadcast_to(bp, (P, O1)).copy(),
        **_iota_inputs(),
    )


def _iota_inputs():
    io8 = np.broadcast_to(np.arange(NCAND, dtype=np.float32),
                          (P, NCAND)).copy()
    io32 = np.broadcast_to(np.arange(4 * NCAND, dtype=np.float32),
                           (P, 4 * NCAND)).copy()
    qoff = np.broadcast_to(
        np.repeat(np.arange(4, dtype=np.float32) * 1024, NCAND),
        (P, 4 * NCAND)).copy()
    return dict(io8=io8, io32=io32, qoff=qoff,
                ones1=np.ones((1, P), np.float32))


def _prep_block2(x1, Wa, Wc, bp):
    xT = np.ascontiguousarray(x1.T)
    bias, selfk = _knn_bias(x1)
    xa = np.zeros((N, XA2), np.float32)
    xa[:, :C2] = x1
    xa[:, C2] = bias
    xa[:, C2 + 1] = selfk
    wch_aug = np.concatenate([Wc, bp[None, :]], axis=0)
    return dict(
        xTr=_xtr_aug(xT, bias),
        x1b_s=_stripes(_bf16(xT), len(_chunks(C2))),
        xa=xa,
        wah=_bf16(Wa),
        wch=_bf16(wch_aug),
        onesb=np.ones((1, P), ml_dtypes.bfloat16),
        **_iota_inputs(),
    )


_LAST_EXEC_NS = {"l1": None, "l2": None}
_X1_DEBUG = {}


def kernel(interm_repr, W1, bn1_gamma, bn1_beta, bn1_mean, bn1_var,
           W2, bn2_gamma, bn2_beta, bn2_mean, bn2_var, _trace=False):
    x = np.asarray(interm_repr, dtype=np.float32)
    p1, p2 = _get_programs()

    W1a, W1c, b1 = _fold_bn(np.asarray(W1), np.asarray(bn1_gamma),
                            np.asarray(bn1_beta), np.asarray(bn1_mean),
                            np.asarray(bn1_var), C1)
    W2a, W2c, b2 = _fold_bn(np.asarray(W2), np.asarray(bn2_gamma),
                            np.asarray(bn2_beta), np.asarray(bn2_mean),
                            np.asarray(bn2_var), C2)

    in_maps = []
    for c in range(8):
        b, h = c // 2, c % 2
        perm = np.r_[h * HALF:(h + 1) * HALF, (1 - h) * HALF:(2 - h) * HALF]
        in_maps.append(_prep_block1(x[b][perm], W1a, W1c, b1))
    r1 = run_bass_kernel_spmd(p1, in_maps, core_ids=list(range(8)), trace=_trace)
    _LAST_EXEC_NS["l1"] = r1.exec_time_ns

    x1 = np.empty((B, N, O1), np.float32)
    for c in range(8):
        b, h = c // 2, c % 2
        x1[b, h * HALF:(h + 1) * HALF] = r1.results[c]["xout"]

    _X1_DEBUG["x1"] = x1
    in_maps = []
    for c in range(8):
        b, h = c // 2, c % 2
        perm = np.r_[h * HALF:(h + 1) * HALF, (1 - h) * HALF:(2 - h) * HALF]
        in_maps.append(_prep_block2(x1[b][perm], W2a, W2c, b2))
    r2 = run_bass_kernel_spmd(p2, in_maps, core_ids=list(range(8)), trace=_trace)
    _LAST_EXEC_NS["l2"] = r2.exec_time_ns

    x2 = np.empty((B, N, O2), np.float32)
    for c in range(8):
        b, h = c // 2, c % 2
        x2[b, h * HALF:(h + 1) * HALF] = r2.results[c]["xout"]
    return x2


if __name__ == "__main__":
    rng = np.random.default_rng(0)
    inp = dict(
        interm_repr=rng.standard_normal((B, N, C1), dtype=np.float32),
        W1=(rng.standard_normal((O1, 2 * C1)) / np.sqrt(2 * C1)).astype(np.float32),
        bn1_gamma=1 + 0.1 * rng.standard_normal(O1).astype(np.float32),
        bn1_beta=0.1 * rng.standard_normal(O1).astype(np.float32),
        bn1_mean=0.1 * rng.standard_normal(O1).astype(np.float32),
        bn1_var=0.5 + rng.random(O1).astype(np.float32),
        W2=(rng.standard_normal((O2, 2 * C2)) / np.sqrt(2 * C2)).astype(np.float32),
        bn2_gamma=1 + 0.1 * rng.standard_normal(O2).astype(np.float32),
        bn2_beta=0.1 * rng.standard_normal(O2).astype(np.float32),
        bn2_mean=0.1 * rng.standard_normal(O2).astype(np.float32),
        bn2_var=0.5 + rng.random(O2).astype(np.float32),
    )
    out = kernel(**inp)
    print("kernel out", out.shape, out.dtype, np.abs(out).mean())
